# revision 112
# baseline (speedup 1.0000x reference)
"""Causal self-attention (RoPE, 16 heads, S=4096, D=1024) on 8 Trainium2 cores.

Sharding: tensor-parallel over heads — core c computes heads 2c, 2c+1.
All matmuls in bf16 (fp32 PSUM accumulate except scores). Per core:
  - q/k projections into [d, s] layout; v via PE transposes into [s, d].
  - RoPE pair-swap via a PE permutation matmul; rotation as
    q*cos + (P@q)*sin with the sign folded into the sin table.
  - Transposed-score attention: scores [k, q] per head; the two heads run
    concurrently on disjoint PE row-groups via tile_position and share one
    exp instruction (N up to 1024). Softmax denominator folds into the PV
    matmul via a ones-column on V; normalization multiplies read PV results
    directly from PSUM (no staging copy).
  - Projection / RoPE / Wo work is sliced into ~2-matmul "pieces" held in
    FIFOs and injected one piece per attention k-tile, so the PE fills its
    stalls between score matmuls with projection work instead of idling
    while ACT computes exp (and ACT never waits behind a monolithic
    projection phase). Wo pieces are held back for the late chunks where
    the projection inventory is exhausted.
  - PV matmuls run TEN k-tiles behind the scores in a queue carried
    ACROSS chunk boundaries: mid-chunk the PE never blocks on exp, a
    chunk's tail PVs overlap the next chunk's scores, each chunk's
    normalize is emitted by its stop=True PV pop, and the next chunk's
    pv banks are allocated lazily right after (clean WAR handoff).
  - A dummy exp at t=0 pulls the one-time ACT table load out of the
    critical path; x-tile DMAs are chained one chunk ahead of use.
  - Row-parallel output projection producing a bf16 partial [S, D];
    host sums the 8 partials in fp32.
"""
import sys
import numpy as np

sys.path.insert(0, "/opt/trn_rl_repo")

import ml_dtypes

import concourse.bacc as bacc
import concourse.mybir as mybir
from concourse.tile import TileContext
from concourse.bass_utils import run_bass_kernel_spmd

FP = mybir.dt.float32
BF = mybir.dt.bfloat16
BF_NP = ml_dtypes.bfloat16

S = 4096          # sequence length
DM = 1024         # model dim
HD = 64           # head dim
NCORES = 8
ROPE_THETA = 10000.0
NQC = 8           # q chunks of 512
QW = 512
NKT = 32          # k tiles of 128
NDC = 8           # d-model chunks of 128

_CACHE = {}


def _build(repeat=1):
    nc = bacc.Bacc("TRN2", target_bir_lowering=False, debug=False,
                   num_devices=NCORES)

    xT = nc.dram_tensor("xT", [DM, S], BF, kind="ExternalInput")
    wq = nc.dram_tensor("wq", [DM, 128], BF, kind="ExternalInput")
    wk = nc.dram_tensor("wk", [DM, 128], BF, kind="ExternalInput")
    wv = nc.dram_tensor("wv", [DM, 128], BF, kind="ExternalInput")
    wo = nc.dram_tensor("wo", [128, DM], BF, kind="ExternalInput")
    cosm = nc.dram_tensor("cosm", [128, S], BF, kind="ExternalInput")
    sinm = nc.dram_tensor("sinm", [128, S], BF, kind="ExternalInput")
    perm = nc.dram_tensor("perm", [128, 128], BF, kind="ExternalInput")
    ident = nc.dram_tensor("ident", [128, 128], BF, kind="ExternalInput")
    OUT = nc.dram_tensor("OUT", [S, DM], BF, kind="ExternalOutput")

    scale = 1.0 / np.sqrt(HD)

    with nc.allow_low_precision(reason="bf16 matmuls within rel-err budget"), \
         TileContext(nc) as tc:
        with tc.tile_pool(name="const", bufs=1) as cpool, \
             tc.tile_pool(name="big", bufs=1) as bpool, \
             tc.tile_pool(name="xt", bufs=5) as xpool, \
             tc.tile_pool(name="pt", bufs=12) as ptpool, \
             tc.tile_pool(name="work", bufs=6) as wpool, \
             tc.tile_pool(name="outp", bufs=4) as opool, \
             tc.tile_pool(name="ps", bufs=1, space="PSUM") as pspool:
          for _rep in range(repeat):
            wq_sb = cpool.tile([128, NDC, 128], BF, tag="wq")
            wk_sb = cpool.tile([128, NDC, 128], BF, tag="wk")
            wv_sb = cpool.tile([128, NDC, 128], BF, tag="wv")
            wo_sb = cpool.tile([128, DM], BF, tag="wo")
            cos_sb = cpool.tile([128, S], BF, tag="cos")
            sin_sb = cpool.tile([128, S], BF, tag="sin")
            pm_sb = cpool.tile([128, 128], BF, tag="perm")
            id_sb = cpool.tile([128, 128], BF, tag="ident")

            # weight shards arrive as [DM, 128] = W_shard.T; stage so chunk dc
            # holds contraction rows dc*128..dc*128+127 on the partition dim.
            # Projection weights + chunk 0's x first so the first matmul isn't
            # queued behind 2.5MB of constants.
            # DMA order tracks first use: the first projection matmuls need
            # only wq/x rows 0:256, so those 320KB go first and the first
            # matmul issues at ~1us instead of ~4us
            xts = {}
            xt0 = xpool.tile([128, NDC, QW], BF, tag="xt")
            xts[0] = xt0
            nc.sync.dma_start(
                wq_sb[:, 0:2, :], wq[0:256, :].rearrange("(c p) e -> p c e",
                                                         p=128))
            nc.sync.dma_start(
                xt0[:, 0:2, :],
                xT[0:256, 0:QW].rearrange("(c p) s -> p c s", p=128))
            nc.sync.dma_start(
                wq_sb[:, 2:8, :], wq[256:1024, :].rearrange("(c p) e -> p c e",
                                                            p=128))
            nc.sync.dma_start(
                xt0[:, 2:4, :],
                xT[256:512, 0:QW].rearrange("(c p) s -> p c s", p=128))
            nc.sync.dma_start(
                xt0[:, 4:8, :],
                xT[512:1024, 0:QW].rearrange("(c p) s -> p c s", p=128))
            nc.sync.dma_start(
                wk_sb[:], wk[:].rearrange("(c p) e -> p c e", p=128))
            nc.sync.dma_start(
                wv_sb[:], wv[:].rearrange("(c p) e -> p c e", p=128))
            nc.sync.dma_start(id_sb[:], ident[:])
            nc.sync.dma_start(pm_sb[:], perm[:])
            nc.sync.dma_start(cos_sb[:, 0:1024], cosm[:, 0:1024])
            nc.sync.dma_start(sin_sb[:, 0:1024], sinm[:, 0:1024])
            # x chunk 1 behind the first-half RoPE tables: proj(1) is FIFO-
            # injected into attn(0) and tolerates the later landing
            xt1 = xpool.tile([128, NDC, QW], BF, tag="xt")
            xts[1] = xt1
            nc.sync.dma_start(
                xt1[:, 0:4, :],
                xT[0:512, QW:2 * QW].rearrange("(c p) s -> p c s", p=128))
            nc.sync.dma_start(
                xt1[:, 4:8, :],
                xT[512:1024, QW:2 * QW].rearrange("(c p) s -> p c s", p=128))
            nc.sync.dma_start(cos_sb[:, 1024:], cosm[:, 1024:])
            nc.sync.dma_start(sin_sb[:, 1024:], sinm[:, 1024:])
            nc.sync.dma_start(wo_sb[:], wo[:])

            # dummy exp with no data deps: the one-time ACT table load
            # (~2.7us on HW) runs during the initial DMA wait
            dum_i = wpool.tile([1, 2], FP, tag="dum_i")
            dum_o = wpool.tile([1, 2], FP, tag="dum_o")
            nc.gpsimd.memset(dum_i[:], 0.0)
            nc.scalar.activation(dum_o[:], dum_i[:],
                                 mybir.ActivationFunctionType.Exp, scale=1.0)

            # PE warm-up spin on zero scratch data: the PE would otherwise
            # idle ~4us waiting for the first weight/x DMAs and then run its
            # first matmuls at the cold HAM clock; these no-dependency
            # matmuls bring it to full rate before real work arrives
            warm = wpool.tile([128, 512], BF, tag="warm")
            nc.gpsimd.memset(warm[:], 0.0)
            warm_ps = pspool.tile([128, 512], FP, tag="s", bufs=2,
                                  name="warm_ps")
            for _ in range(9):
                nc.tensor.matmul(warm_ps[:], warm[:, 0:128], warm[:],
                                 start=True, stop=True)

            q_sb = bpool.tile([128, S], BF, tag="q")
            k_sb = bpool.tile([128, S], BF, tag="k")
            v_sb = bpool.tile([128, NKT, 130], BF, tag="v")
            o_sb = bpool.tile([128, S], BF, tag="o")

            # ones columns for the softmax-denominator rows of the PV matmuls
            nc.gpsimd.memset(v_sb[:, :, 64:65], 1.0)
            nc.gpsimd.memset(v_sb[:, :, 129:130], 1.0)

            def emit_xt_dma(sc):
                if sc > NQC - 1 or sc in xts:
                    return
                t = xpool.tile([128, NDC, QW], BF, tag="xt")
                xts[sc] = t
                nc.sync.dma_start(
                    t[:], xT[:, sc * QW:(sc + 1) * QW]
                    .rearrange("(c p) s -> p c s", p=128))

            # ---- projection pieces (chunk sc), split in ~2-matmul quarters
            # so one piece matches a single attention unit's PE slack:
            # q0..q3 rope-q k0..k3 rope-k v0..v3 vT0 vT1
            vt_tmps = {}
            proj_ps = {}

            def piece_proj(sc, w_sb, dst_sel, qi):
                def run():
                    if dst_sel == "q" and qi == 0:
                        emit_xt_dma(sc + 1)
                    ssl = slice(sc * QW, (sc + 1) * QW)
                    xt = xts[sc]
                    if qi == 0:
                        psp = pspool.tile([128, QW], FP, tag="mm", bufs=2)
                        proj_ps[(sc, dst_sel)] = psp
                    else:
                        psp = proj_ps[(sc, dst_sel)]
                    for dc in range(qi * 2, qi * 2 + 2):
                        nc.tensor.matmul(psp[:], w_sb[:, dc, :], xt[:, dc, :],
                                         start=(dc == 0), stop=(dc == NDC - 1))
                    if qi < 3:
                        return
                    del proj_ps[(sc, dst_sel)]
                    # early chunks evacuate via ACT (idle until attention
                    # ramps) so slot recycling never queues behind DVE
                    cp = (nc.scalar.copy if sc in (2, 3) else
                          lambda o, i: nc.vector.tensor_copy(o, i))
                    if dst_sel == "q":
                        cp(q_sb[:, ssl], psp[:])
                    elif dst_sel == "k":
                        cp(k_sb[:, ssl], psp[:])
                    else:
                        vt = wpool.tile([128, QW], BF, tag="vt")
                        vt_tmps[sc] = vt
                        cp(vt[:], psp[:])
                return run

            def piece_vt(sc, half):
                def run():
                    vt = vt_tmps[sc]
                    cp = (nc.scalar.copy if sc in (2, 3) else
                          lambda o, i: nc.vector.tensor_copy(o, i))
                    for j in range(half * 2, half * 2 + 2):
                        kt = 4 * sc + j
                        pst = pspool.tile([128, 128], BF, tag="mm", bufs=2)
                        nc.tensor.transpose(pst[:], vt[:, j * 128:(j + 1) * 128],
                                            id_sb[:])
                        cp(v_sb[:, kt, 0:64], pst[:, 0:64])
                        cp(v_sb[:, kt, 65:129], pst[:, 64:128])
                    if half == 1:
                        del vt_tmps[sc]
                return run

            def piece_rope(sc, t_sb):
                # t = t*cos + (P@t)*sin (sign baked into sin)
                def run():
                    ssl = slice(sc * QW, (sc + 1) * QW)
                    psw = pspool.tile([128, QW], FP, tag="mm", bufs=2)
                    nc.tensor.matmul(psw[:], pm_sb[:], t_sb[:, ssl],
                                     start=True, stop=True)
                    t1 = wpool.tile([128, QW], BF, tag="t1")
                    t2 = wpool.tile([128, QW], BF, tag="t2")
                    nc.vector.tensor_tensor(t1[:], t_sb[:, ssl], cos_sb[:, ssl],
                                            mybir.AluOpType.mult)
                    nc.vector.tensor_tensor(t2[:], psw[:], sin_sb[:, ssl],
                                            mybir.AluOpType.mult)
                    nc.vector.tensor_tensor(t_sb[:, ssl], t1[:], t2[:],
                                            mybir.AluOpType.add)
                return run

            def proj_pieces(sc):
                # rope directly after its projection: the rope PSUM slot is
                # then freed by small pst copies (not a queued DVE chain) by
                # the time the next chunk's first projection wants it
                out = []
                for qi in range(4):
                    out.append((sc, piece_proj(sc, wq_sb, "q", qi)))
                out.append((sc, piece_rope(sc, q_sb)))
                for qi in range(4):
                    out.append((sc, piece_proj(sc, wk_sb, "k", qi)))
                out.append((sc, piece_rope(sc, k_sb)))
                for qi in range(4):
                    out.append((sc, piece_proj(sc, wv_sb, "v", qi)))
                out.append((sc, piece_vt(sc, 0)))
                out.append((sc, piece_vt(sc, 1)))
                return out

            # ---- output-projection pieces (chunk qc, s-tile j2, e-half eh)
            ot_tiles = {}

            wo_pf2 = {}

            def piece_wo(qc, j2, eh):
                def run():
                    st = qc * 4 + j2
                    if eh == 0:
                        ot = opool.tile([128, DM], BF, tag="ot")
                        ot_tiles[st] = ot
                    else:
                        ot = ot_tiles.pop(st)
                    if qc == NQC - 1:
                        # tail: the score pool's banks are free once the last
                        # exp is consumed — borrow a double-width tile so both
                        # e-halves evacuate in ONE copy, on the then-idle ACT
                        if eh == 0:
                            pf2 = pspool.tile([128, 1024], FP, tag="s",
                                              bufs=2, name="pf2")
                            wo_pf2[st] = pf2
                        else:
                            pf2 = wo_pf2.pop(st)
                        nc.tensor.matmul(
                            pf2[:, eh * 512:(eh + 1) * 512],
                            o_sb[:, st * 128:(st + 1) * 128],
                            wo_sb[:, eh * 512:(eh + 1) * 512],
                            start=True, stop=True)
                        if eh == 1:
                            nc.scalar.copy(ot[:], pf2[:])
                            nc.sync.dma_start(
                                OUT[st * 128:(st + 1) * 128, :], ot[:])
                        return
                    pf = pspool.tile([128, QW], FP, tag="mm", bufs=2)
                    nc.tensor.matmul(
                        pf[:], o_sb[:, st * 128:(st + 1) * 128],
                        wo_sb[:, eh * 512:(eh + 1) * 512],
                        start=True, stop=True)
                    nc.vector.tensor_copy(
                        ot[:, eh * 512:(eh + 1) * 512], pf[:])
                    if eh == 1:
                        nc.sync.dma_start(OUT[st * 128:(st + 1) * 128, :],
                                          ot[:])
                return run

            proj_fifo = []
            wo_fifo = []

            def pop_piece(qc, kt):
                # proj pieces ASAP (they gate attention chunks); Wo pieces
                # are deadline-free so hold them for the late chunks where
                # the proj inventory is exhausted and PE would otherwise
                # idle ~190ns/unit behind ACT's exp cadence
                if proj_fifo:
                    proj_fifo.pop(0)[1]()
                elif wo_fifo and qc >= 5:
                    wo_fifo.pop(0)()

            # ---- attention chunk qc: scores [k, q] per head, both heads
            # packed on disjoint PE row-groups; PV pipelined well behind the
            # scores/exp stream; proj/Wo pieces injected per k-tile
            pend = []
            norm_fns = {}

            def attn_chunk(qc):
                qsl = slice(qc * QW, (qc + 1) * QW)
                nkt = 4 * (qc + 1)
                # pv banks allocated lazily at this chunk's FIRST PV emission
                # (which happens AFTER the previous chunk's normalize in the
                # global pend order, giving a clean WAR handoff of the banks)
                pvt = []

                def get_pvs():
                    if not pvt:
                        pvt.append(pspool.tile([65, QW], FP, tag="pv0",
                                               bufs=1, name="pv0"))
                        pvt.append(pspool.tile([65, QW], FP, tag="pv1",
                                               bufs=1, name="pv1"))
                    return pvt

                def mk_pv(kt, pt, coff):
                    # suffix-only accumulate is safe: diagonal tiles are
                    # never the start=True tile unless coff == 0
                    def emit():
                        pv0, pv1 = get_pvs()
                        nc.tensor.matmul(pv0[:, coff:], v_sb[:, kt, 0:65],
                                         pt[:, 0, coff:],
                                         start=(kt == 0),
                                         stop=(kt == nkt - 1))
                        nc.tensor.matmul(pv1[:, coff:], v_sb[:, kt, 65:130],
                                         pt[:, 1, coff:],
                                         start=(kt == 0),
                                         stop=(kt == nkt - 1))
                        if kt == nkt - 1:
                            # chunk complete: emit its normalize right here,
                            # before the next chunk's first PV (next in the
                            # pend queue) reuses the pv banks
                            norm_fns.pop(qc)()
                    return emit

                for kt in range(nkt):
                    pop_piece(qc, kt)
                    ksl = slice(kt * 128, (kt + 1) * 128)
                    coff = (kt - 4 * qc) * 128 if kt > 4 * qc else 0
                    qs0 = qc * QW + coff
                    ps_s = pspool.tile([128, 1024], FP, tag="s", bufs=2)
                    ps3 = ps_s[:].rearrange("p (h q) -> p h q", h=2)
                    nc.tensor.matmul(ps3[:, 0, coff:],
                                     k_sb[0:64, ksl],
                                     q_sb[0:64, qs0:(qc + 1) * QW],
                                     start=True, stop=True,
                                     tile_position=(0, 0))
                    nc.tensor.matmul(ps3[:, 1, coff:],
                                     k_sb[64:128, ksl],
                                     q_sb[64:128, qs0:(qc + 1) * QW],
                                     start=True, stop=True,
                                     tile_position=(64, 0))
                    pt = ptpool.tile([128, 2, 512], BF, tag="pt")
                    # diagonal tiles: columns < coff are fully above the
                    # causal boundary — skip their exp; the affine_select
                    # below writes fill=0 over that whole region anyway
                    nc.scalar.activation(pt[:, :, coff:], ps3[:, :, coff:],
                                         mybir.ActivationFunctionType.Exp,
                                         scale=scale)
                    if kt >= 4 * qc:  # diagonal tile: zero where k > q
                        for h in range(2):
                            nc.gpsimd.affine_select(
                                out=pt[:, h, coff:],
                                in_=pt[:, h, coff:],
                                compare_op=mybir.AluOpType.is_ge,
                                fill=0.0, base=0,
                                pattern=[[1, 512 - coff]],
                                channel_multiplier=-1)
                    # PV runs SEVEN k-tiles behind the scores, carried ACROSS
                    # chunk boundaries: mid-chunk the PE never blocks on exp
                    # (long done), and a chunk's tail PVs overlap the next
                    # chunk's score/piece work instead of flushing in a
                    # stalling burst at the boundary
                    pend.append(mk_pv(kt, pt, coff))
                    if len(pend) > 10:
                        pend.pop(0)()

                # normalize straight out of PSUM (no staging copy); emitted
                # from the pend queue right after this chunk's stop=True PV
                def normalize():
                    pv0, pv1 = get_pvs()
                    if qc < NQC - 1:
                        r_sb = wpool.tile([1, 1024], FP, tag="r")
                        nc.vector.reciprocal(r_sb[0:1, 0:512], pv0[64:65, :])
                        nc.vector.reciprocal(r_sb[0:1, 512:1024],
                                             pv1[64:65, :])
                        for h, pv in ((0, pv0), (1, pv1)):
                            bc = wpool.tile([64, QW], FP, tag="bc")
                            nc.gpsimd.partition_broadcast(
                                bc[:], r_sb[0:1, h * 512:(h + 1) * 512],
                                channels=64)
                            nc.vector.tensor_tensor(
                                o_sb[h * 64:(h + 1) * 64, qsl], pv[0:64, :],
                                bc[:], mybir.AluOpType.mult)
                        for j2 in range(4):
                            for eh in range(2):
                                wo_fifo.append(piece_wo(qc, j2, eh))
                        return
                    # last chunk: normalize per 128-column quarter and chase
                    # each quarter with its Wo s-tile, so the tail chain is
                    # recip[128]->bcast->mult->Wo instead of the full-width
                    # serial chain before any Wo work
                    for j2 in range(4):
                        csl = slice(j2 * 128, (j2 + 1) * 128)
                        qsl4 = slice(qc * QW + j2 * 128,
                                     qc * QW + (j2 + 1) * 128)
                        r_sb = wpool.tile([1, 256], FP, tag="r")
                        nc.vector.reciprocal(r_sb[0:1, 0:128], pv0[64:65, csl])
                        nc.vector.reciprocal(r_sb[0:1, 128:256],
                                             pv1[64:65, csl])
                        for h, pv in ((0, pv0), (1, pv1)):
                            bc = wpool.tile([64, 128], FP, tag="bc")
                            nc.gpsimd.partition_broadcast(
                                bc[:], r_sb[0:1, h * 128:(h + 1) * 128],
                                channels=64)
                            nc.vector.tensor_tensor(
                                o_sb[h * 64:(h + 1) * 64, qsl4],
                                pv[0:64, csl], bc[:], mybir.AluOpType.mult)
                        for eh in range(2):
                            piece_wo(qc, j2, eh)()
                norm_fns[qc] = normalize

            # ---- main schedule: proj(0), proj(1) up front with ropes last
            # (their cos/sin tables are at the tail of the DMA queue);
            # proj(2..7) and Wo(*) injected one piece per attention unit
            p0 = proj_pieces(0)
            ROPE_IDX = (4, 9)
            for i, (_, piece) in enumerate(p0):
                if i not in ROPE_IDX:
                    piece()
            for i in ROPE_IDX:
                p0[i][1]()
            proj_fifo.extend(proj_pieces(1))
            for sc in range(2, NQC):
                proj_fifo.extend(proj_pieces(sc))

            for qc in range(NQC):
                while proj_fifo and proj_fifo[0][0] <= qc:
                    proj_fifo.pop(0)[1]()
                attn_chunk(qc)
            while pend:
                pend.pop(0)()
            while wo_fifo:
                wo_fifo.pop(0)()

    nc.compile()
    return nc


def _host_prep(x, Wq, Wk, Wv, Wo):
    x = np.asarray(x, dtype=np.float32)
    Wq = np.asarray(Wq, dtype=np.float32)
    Wk = np.asarray(Wk, dtype=np.float32)
    Wv = np.asarray(Wv, dtype=np.float32)
    Wo = np.asarray(Wo, dtype=np.float32)

    xT = np.ascontiguousarray(x.reshape(S, DM).T).astype(BF_NP)

    # RoPE tables in the [d, s] layout (sign of the swap folded into sin)
    pos = np.arange(S, dtype=np.float32)
    inv_freq = (ROPE_THETA ** (-np.arange(0, HD, 2, dtype=np.float32) / HD))
    ang = pos[None, :] * inv_freq[:, None]          # [32, S]
    cos_p = np.cos(ang).astype(np.float32)
    sin_p = np.sin(ang).astype(np.float32)
    cosm = np.empty((128, S), np.float32)
    sinm = np.empty((128, S), np.float32)
    for h in range(2):
        b = h * HD
        cosm[b + 0:b + HD:2] = cos_p
        cosm[b + 1:b + HD:2] = cos_p
        sinm[b + 0:b + HD:2] = -sin_p
        sinm[b + 1:b + HD:2] = sin_p
    cosm = cosm.astype(BF_NP)
    sinm = sinm.astype(BF_NP)

    # pair-swap permutation: P[i, j] = 1 iff i == j ^ 1 (within each head)
    perm = np.zeros((128, 128), np.float32)
    idx = np.arange(128)
    perm[idx ^ 1, idx] = 1.0
    perm = perm.astype(BF_NP)
    identm = np.eye(128, dtype=np.float32).astype(BF_NP)

    in_maps = []
    for c in range(NCORES):
        rows = slice(128 * c, 128 * (c + 1))
        in_maps.append({
            "xT": xT,
            "wq": np.ascontiguousarray(Wq[rows, :].T).astype(BF_NP),
            "wk": np.ascontiguousarray(Wk[rows, :].T).astype(BF_NP),
            "wv": np.ascontiguousarray(Wv[rows, :].T).astype(BF_NP),
            "wo": np.ascontiguousarray(Wo[:, rows].T).astype(BF_NP),
            "cosm": cosm,
            "sinm": sinm,
            "perm": perm,
            "ident": identm,
        })
    return in_maps


def kernel(x, Wq, Wk, Wv, Wo, _trace=False, _trace_kwargs=None):
    if "nc" not in _CACHE:
        _CACHE["nc"] = _build()
    nc = _CACHE["nc"]
    in_maps = _host_prep(x, Wq, Wk, Wv, Wo)
    kw = {}
    if _trace:
        kw = dict(trace=True, **(_trace_kwargs or {}))
    res = run_bass_kernel_spmd(nc, in_maps, core_ids=list(range(NCORES)), **kw)
    out = np.zeros((S, DM), np.float32)
    for r in res.results:
        out += np.asarray(r["OUT"], dtype=np.float32)
    _CACHE["last_results"] = res
    return out.astype(np.float32).reshape(1, S, DM)
